# revision 20
# baseline (speedup 1.0000x reference)
"""Trainium2 Bass kernel for nn_DecoderAllSentinel (ragged decoder).

Math: for each ragged token (i, t):
    recons[n] = transpose((x[i] + table[dates[i,t]]) @ W.T + b)   # [C, P]
Distributing the matmul:
    recons[n, c, p] = XW[i, c, p] + EWc[dates[i, t], c]
with XW[i] = (x[i] @ W.T).T (computed on device, PE) and
EWc = table @ W.T + b (folded host-side: all-constant weights).

Sharding: data-parallel over the batch dim, 8 batches per core.  Each core
computes a padded [8*256, 640] output (every batch padded to TMAX tokens,
<1% waste); the host slices each batch's ragged prefix and concatenates.

Device pipeline per core:
  - 8 indirect gathers fetch EWc rows for 2048 tokens (a host-built
    date-pair table [367^2, 20] serves 2 tokens per index, halving the
    SWDGE descriptor-emission cost, which is the serial bottleneck).
  - x is PE-transposed, XW = W @ x[i].T on PE, flattened to [1, 640] rows
    by one SBUF->SBUF DMA, then replicated across 128 partitions by a
    K=1 ones-matmul into PSUM.
  - One DVE add per 128-token tile (PSUM XW-broadcast + E free-broadcast)
    writes the output tile; one HWDGE store per tile (327 KB).
"""

import os
import sys

import numpy as np

try:
    import concourse.bacc as bacc
except ImportError:  # fresh env: fall back to the in-container repo path
    for p in ("/opt/trn_rl_repo", "/root/.axon_site/_ro/trn_rl_repo"):
        if os.path.isdir(p) and p not in sys.path:
            sys.path.insert(0, p)
    import concourse.bacc as bacc
import concourse.bass as bass
import concourse.mybir as mybir
import concourse.tile as tile
from concourse import bass_utils

B, TMAX, P, D, C, TBASE = 64, 256, 64, 128, 10, 367
NCORES = 8
BPC = B // NCORES          # 8 batches per core
F = C * P                  # 640 output floats per token
NTILES = BPC * (TMAX // 128)   # 16 column tiles of 128 tokens
NPAIR = NTILES // 2        # 8 pair-gathers (2 tiles per gather)

F32 = mybir.dt.float32
I32 = mybir.dt.int32

_cache: dict = {}
LAST_RESULTS = None  # BassKernelResults of the most recent run (for test harness)


def _sinusoid_table():
    pos = np.arange(TBASE, dtype=np.float64)[:, None]
    j = np.arange(D)[None, :]
    ang = pos / np.power(float(TBASE), 2.0 * (j // 2) / D)
    tab = ang.copy()
    tab[:, 0::2] = np.sin(ang[:, 0::2])
    tab[:, 1::2] = np.cos(ang[:, 1::2])
    return tab.astype(np.float32)


def _build_program():
    nc = bacc.Bacc("TRN2", target_bir_lowering=False, debug=False)

    x_d = nc.dram_tensor("x_s", [BPC, P, D], F32, kind="ExternalInput").ap()
    ewc2_d = nc.dram_tensor("ewc2", [TBASE * TBASE, 2 * C], F32,
                            kind="ExternalInput").ap()
    idx2_d = nc.dram_tensor("idx2", [128, NPAIR], I32, kind="ExternalInput").ap()
    # aux packs W.T [128,10] | iden2 [128,64] | ones row [1,128] (partition 0)
    aux_d = nc.dram_tensor("aux", [128, 10 + P + 128], F32,
                           kind="ExternalInput").ap()
    out_d = nc.dram_tensor("out", [BPC * TMAX, F], F32, kind="ExternalOutput").ap()

    # out rows are i*TMAX + h*128 + t ; per-batch view enumerated (t, h, f)
    out_r = out_d.rearrange("(i h t) f -> i t h f", i=BPC, h=2, t=128)

    with tile.TileContext(nc) as tc:
        with (
            tc.tile_pool(name="const", bufs=1) as cpool,
            tc.tile_pool(name="outp", bufs=8) as opool,
            tc.tile_pool(name="psum", bufs=4, space="PSUM") as ppool,
        ):
            x2_sb = cpool.tile([128, (BPC // 2) * D], F32, tag="x2")
            nc.sync.dma_start(
                x2_sb[:], x_d.rearrange("(a b) p d -> b p a d", a=BPC // 2, b=2)
            )
            aux_sb = cpool.tile([128, 10 + P + 128], F32, tag="aux")
            nc.sync.dma_start(aux_sb[:], aux_d[:])
            wt_sb = aux_sb[:, 0:C]                      # [128, 10]
            iden_sb = aux_sb[:, C:C + P]                # [128, 64] (eye x2)
            ones_sb = aux_sb[0:1, C + P:C + P + 128]    # [1, 128]
            idx_sb = cpool.tile([128, NPAIR], I32, tag="idx")
            nc.sync.dma_start(idx_sb[:], idx2_d[:])

            e_sbs = []
            for g in range(NPAIR):
                e_sb = cpool.tile([128, 2 * C], F32, tag=f"e{g}")
                nc.gpsimd.indirect_dma_start(
                    out=e_sb[:],
                    out_offset=None,
                    in_=ewc2_d[:],
                    in_offset=bass.IndirectOffsetOnAxis(
                        ap=idx_sb[:, g:g + 1], axis=0
                    ),
                )
                e_sbs.append(e_sb)

            xt_sb = cpool.tile([D, BPC * P], F32, tag="xt")
            xw_sb = cpool.tile([C, BPC * P], F32, tag="xw")
            xwrow_sb = cpool.tile([1, BPC * F], F32, tag="xwrow")
            # xwrow free layout is (c, i_local, p) per prologue group
            GB = 2          # batches per prologue group
            xwrow_q = [
                xwrow_sb[0:1, q * GB * F:(q + 1) * GB * F].rearrange(
                    "o (c i p) -> o c i p", c=C, i=GB, p=P)
                for q in range(BPC // GB)
            ]

            # pipelined prologue groups: transpose GB batches -> XW -> flatten
            def prologue_group(q):
                for i in range(q * GB, (q + 1) * GB):
                    a, bb = i // 2, i % 2
                    pt = ppool.tile([D, P], F32, tag="ps")
                    nc.tensor.transpose(
                        out=pt[:],
                        in_=x2_sb[bb * P:(bb + 1) * P, a * D:(a + 1) * D],
                        identity=iden_sb[bb * P:(bb + 1) * P, :],
                    )
                    nc.vector.tensor_copy(
                        out=xt_sb[:, i * P:(i + 1) * P], in_=pt[:])
                pxw = ppool.tile([C, GB * P], F32, tag="ps")
                nc.tensor.matmul(
                    out=pxw[:], lhsT=wt_sb,
                    rhs=xt_sb[:, q * GB * P:(q + 1) * GB * P],
                    start=True, stop=True)
                nc.vector.tensor_copy(
                    out=xw_sb[:, q * GB * P:(q + 1) * GB * P], in_=pxw[:])
                # flatten group q: (c, i, p) on both sides
                nc.sync.dma_start(out=xwrow_q[q],
                                  in_=xw_sb[:, q * GB * P:(q + 1) * GB * P])

            def batch_work(i):
                q, il = i // GB, i % GB
                pb = ppool.tile([128, F], F32, tag="ps")
                nc.tensor.matmul(out=pb[:, 0:512], lhsT=ones_sb,
                                 rhs=xwrow_q[q][:, 0:8, il, :],
                                 start=True, stop=True)
                nc.tensor.matmul(out=pb[:, 512:F], lhsT=ones_sb,
                                 rhs=xwrow_q[q][:, 8:10, il, :],
                                 start=True, stop=True)
                for h in range(2):
                    ot = opool.tile([128, F], F32, tag="out")
                    esl = e_sbs[i][:, h * C:(h + 1) * C, None].to_broadcast(
                        (128, C, P)
                    )
                    nc.vector.tensor_tensor(
                        out=ot[:].rearrange("t (c p) -> t c p", c=C, p=P),
                        in0=pb[:, :].rearrange("t (c p) -> t c p", c=C, p=P),
                        in1=esl,
                        op=mybir.AluOpType.add,
                    )
                    nc.sync.dma_start(out=out_r[i][:, h, :], in_=ot[:])

            for q in range(BPC // GB):
                prologue_group(q)
            for i in range(BPC):
                batch_work(i)

    nc.compile()
    return nc


def kernel(x, attentions, dates, W, b):
    global LAST_RESULTS
    x = np.ascontiguousarray(np.asarray(x, dtype=np.float32))
    dates = np.ascontiguousarray(np.asarray(dates, dtype=np.int32))
    W = np.asarray(W, dtype=np.float32)
    b = np.asarray(b, dtype=np.float32)

    # ragged lengths (same formula as the reference; host-side metadata)
    nz = dates != 0
    lengths = dates.shape[1] - 1 - np.argmax(nz[:, ::-1], axis=1)

    # fold the frozen sinusoid table through the decode weights (constants)
    tab = _sinusoid_table()
    ewc = (tab.astype(np.float64) @ W.astype(np.float64).T
           + b.astype(np.float64)).astype(np.float32)
    # date-pair table: row d1*TBASE+d2 = [EWc[d1] | EWc[d2]]
    ewc2 = np.empty((TBASE * TBASE, 2 * C), np.float32)
    ewc2[:, :C] = np.repeat(ewc, TBASE, axis=0)
    ewc2[:, C:] = np.tile(ewc, (TBASE, 1))

    aux = np.zeros((128, 10 + P + 128), np.float32)
    aux[:, 0:C] = W.T
    aux[:, C:C + P] = np.tile(np.eye(P, dtype=np.float32), (2, 1))
    aux[0, C + P:] = 1.0

    if "prog" not in _cache:
        _cache["prog"] = _build_program()
    nc = _cache["prog"]

    in_maps = []
    for core in range(NCORES):
        ds = dates[core * BPC:(core + 1) * BPC]
        idx = ds.reshape(BPC, 2, 128).transpose(2, 0, 1).reshape(128, NTILES)
        idx2 = np.ascontiguousarray(
            idx[:, 0::2].astype(np.int64) * TBASE + idx[:, 1::2]
        ).astype(np.int32)
        in_maps.append({
            "x_s": np.ascontiguousarray(x[core * BPC:(core + 1) * BPC]),
            "ewc2": ewc2,
            "idx2": idx2,
            "aux": aux,
        })

    res = bass_utils.run_bass_kernel_spmd(
        nc, in_maps, core_ids=list(range(NCORES)),
        trace=bool(os.environ.get("BASS_TRACE")),
    )
    LAST_RESULTS = res

    # unshard: slice each batch's ragged prefix from the padded output
    pieces = []
    for core in range(NCORES):
        o = res.results[core]["out"].reshape(BPC, TMAX, C, P)
        for i in range(BPC):
            pieces.append(o[i, :lengths[core * BPC + i]])
    recons = np.concatenate(pieces, axis=0)

    batch_idx = np.repeat(np.arange(B), lengths)
    time_idx = np.concatenate([np.arange(L) for L in lengths])
    masks = np.stack([batch_idx.astype(np.int32), time_idx.astype(np.int32)],
                     axis=1)
    return recons, masks


# revision 22
# speedup vs baseline: 1.0770x; 1.0770x over previous
"""Trainium2 Bass kernel for nn_DecoderAllSentinel (ragged decoder).

Math: for each ragged token (i, t):
    recons[n] = transpose((x[i] + table[dates[i,t]]) @ W.T + b)   # [C, P]
Distributing the matmul:
    recons[n, c, p] = XW[i, c, p] + EWc[dates[i, t], c]
with XW[i] = (x[i] @ W.T).T (computed on device, PE) and
EWc = table @ W.T + b (folded host-side: all-constant weights).

Sharding: data-parallel over the batch dim, 8 batches per core.  Each core
computes a padded [8*256, 640] output (every batch padded to TMAX tokens,
<1% waste); the host slices each batch's ragged prefix and concatenates.

Device pipeline per core:
  - 8 indirect gathers fetch EWc rows for 2048 tokens (a host-built
    date-pair table [367^2, 20] serves 2 tokens per index, halving the
    SWDGE descriptor-emission cost, which is the serial bottleneck).
  - x is PE-transposed, XW = W @ x[i].T on PE, flattened to [1, 640] rows
    by one SBUF->SBUF DMA, then replicated across 128 partitions by a
    K=1 ones-matmul into PSUM.
  - One DVE add per 128-token tile (PSUM XW-broadcast + E free-broadcast)
    writes the output tile; one HWDGE store per tile (327 KB).
"""

import os
import sys

import numpy as np

try:
    import concourse.bacc as bacc
except ImportError:  # fresh env: fall back to the in-container repo path
    for p in ("/opt/trn_rl_repo", "/root/.axon_site/_ro/trn_rl_repo"):
        if os.path.isdir(p) and p not in sys.path:
            sys.path.insert(0, p)
    import concourse.bacc as bacc
import concourse.bass as bass
import concourse.mybir as mybir
import concourse.tile as tile
from concourse import bass_utils

B, TMAX, P, D, C, TBASE = 64, 256, 64, 128, 10, 367
NCORES = 8
BPC = B // NCORES          # 8 batches per core
F = C * P                  # 640 output floats per token
NTILES = BPC * (TMAX // 128)   # 16 column tiles of 128 tokens
NPAIR = NTILES // 2        # 8 pair-gathers (2 tiles per gather)

F32 = mybir.dt.float32
I32 = mybir.dt.int32

_cache: dict = {}
LAST_RESULTS = None  # BassKernelResults of the most recent run (for test harness)


def _sinusoid_table():
    pos = np.arange(TBASE, dtype=np.float64)[:, None]
    j = np.arange(D)[None, :]
    ang = pos / np.power(float(TBASE), 2.0 * (j // 2) / D)
    tab = ang.copy()
    tab[:, 0::2] = np.sin(ang[:, 0::2])
    tab[:, 1::2] = np.cos(ang[:, 1::2])
    return tab.astype(np.float32)


def _build_program():
    nc = bacc.Bacc("TRN2", target_bir_lowering=False, debug=False)

    x_d = nc.dram_tensor("x_s", [BPC, P, D], F32, kind="ExternalInput").ap()
    ewc2_d = nc.dram_tensor("ewc2", [TBASE * TBASE, 2 * C], F32,
                            kind="ExternalInput").ap()
    idx2_d = nc.dram_tensor("idx2", [128, NPAIR], I32, kind="ExternalInput").ap()
    # aux packs W.T [128,10] | iden2 [128,64] | ones row [1,128] (partition 0)
    aux_d = nc.dram_tensor("aux", [128, 10 + P + 128], F32,
                           kind="ExternalInput").ap()
    out_d = nc.dram_tensor("out", [BPC * TMAX, F], F32, kind="ExternalOutput").ap()

    # out rows are i*TMAX + h*128 + t ; per-batch view enumerated (t, h, f)
    out_r = out_d.rearrange("(i h t) f -> i t h f", i=BPC, h=2, t=128)

    with tile.TileContext(nc) as tc:
        with (
            tc.tile_pool(name="const", bufs=1) as cpool,
            tc.tile_pool(name="outp", bufs=8) as opool,
            tc.tile_pool(name="psum", bufs=4, space="PSUM") as ppool,
        ):
            # per-group x loads: group q (batches 2q, 2q+1) only needs its
            # own [128, 128] slab, so its chain starts as soon as that lands
    # (x viewed as (a b) p d -> (b p)(d) per pair a=q)
            x_r = x_d.rearrange("(a b) p d -> a b p d", a=BPC // 2, b=2)
            x2q_sb = []
            x2q_sb.append(cpool.tile([128, D], F32, tag="x2q0", name="x2q0"))
            nc.sync.dma_start(x2q_sb[0][:], x_r[0])
            aux_sb = cpool.tile([128, 10 + P + 128], F32, tag="aux")
            nc.sync.dma_start(aux_sb[:], aux_d[:])
            wt_sb = aux_sb[:, 0:C]                      # [128, 10]
            iden_sb = aux_sb[:, C:C + P]                # [128, 64] (eye x2)
            ones_sb = aux_sb[0:1, C + P:C + P + 128]    # [1, 128]
            x2q_sb.append(cpool.tile([128, D], F32, tag="x2q1", name="x2q1"))
            nc.sync.dma_start(x2q_sb[1][:], x_r[1])
            idx_sb = cpool.tile([128, NPAIR], I32, tag="idx")
            nc.sync.dma_start(idx_sb[:], idx2_d[:])
            for qq in range(2, BPC // 2):
                x2q_sb.append(cpool.tile([128, D], F32, tag=f"x2q{qq}", name=f"x2q{qq}"))
                nc.sync.dma_start(x2q_sb[qq][:], x_r[qq])

            e_sbs = []
            for g in range(NPAIR):
                e_sb = cpool.tile([128, 2 * C], F32, tag=f"e{g}")
                nc.gpsimd.indirect_dma_start(
                    out=e_sb[:],
                    out_offset=None,
                    in_=ewc2_d[:],
                    in_offset=bass.IndirectOffsetOnAxis(
                        ap=idx_sb[:, g:g + 1], axis=0
                    ),
                )
                e_sbs.append(e_sb)

            xt_sb = cpool.tile([D, BPC * P], F32, tag="xt")
            xw_sb = cpool.tile([C, BPC * P], F32, tag="xw")
            xwrow_sb = cpool.tile([1, BPC * F], F32, tag="xwrow")
            # xwrow free layout is (c, i_local, p) per prologue group
            GB = 2          # batches per prologue group
            xwrow_q = [
                xwrow_sb[0:1, q * GB * F:(q + 1) * GB * F].rearrange(
                    "o (c i p) -> o c i p", c=C, i=GB, p=P)
                for q in range(BPC // GB)
            ]

            # pipelined prologue groups: transpose GB batches -> XW -> flatten
            def prologue_group(q):
                for i in range(q * GB, (q + 1) * GB):
                    a, bb = i // 2, i % 2
                    pt = ppool.tile([D, P], F32, tag="ps")
                    nc.tensor.transpose(
                        out=pt[:],
                        in_=x2q_sb[a][bb * P:(bb + 1) * P, :],
                        identity=iden_sb[bb * P:(bb + 1) * P, :],
                    )
                    nc.vector.tensor_copy(
                        out=xt_sb[:, i * P:(i + 1) * P], in_=pt[:])
                pxw = ppool.tile([C, GB * P], F32, tag="ps")
                nc.tensor.matmul(
                    out=pxw[:], lhsT=wt_sb,
                    rhs=xt_sb[:, q * GB * P:(q + 1) * GB * P],
                    start=True, stop=True)
                nc.vector.tensor_copy(
                    out=xw_sb[:, q * GB * P:(q + 1) * GB * P], in_=pxw[:])
                # flatten group q: (c, i, p) on both sides
                nc.sync.dma_start(out=xwrow_q[q],
                                  in_=xw_sb[:, q * GB * P:(q + 1) * GB * P])

            def batch_work(i):
                q, il = i // GB, i % GB
                pb = ppool.tile([128, F], F32, tag="ps")
                nc.tensor.matmul(out=pb[:, 0:512], lhsT=ones_sb,
                                 rhs=xwrow_q[q][:, 0:8, il, :],
                                 start=True, stop=True)
                nc.tensor.matmul(out=pb[:, 512:F], lhsT=ones_sb,
                                 rhs=xwrow_q[q][:, 8:10, il, :],
                                 start=True, stop=True)
                for h in range(2):
                    ot = opool.tile([128, F], F32, tag="out")
                    esl = e_sbs[i][:, h * C:(h + 1) * C, None].to_broadcast(
                        (128, C, P)
                    )
                    nc.vector.tensor_tensor(
                        out=ot[:].rearrange("t (c p) -> t c p", c=C, p=P),
                        in0=pb[:, :].rearrange("t (c p) -> t c p", c=C, p=P),
                        in1=esl,
                        op=mybir.AluOpType.add,
                    )
                    nc.sync.dma_start(out=out_r[i][:, h, :], in_=ot[:])

            for q in range(BPC // GB):
                prologue_group(q)
            for i in range(BPC):
                batch_work(i)

    nc.compile()
    return nc


def kernel(x, attentions, dates, W, b):
    global LAST_RESULTS
    x = np.ascontiguousarray(np.asarray(x, dtype=np.float32))
    dates = np.ascontiguousarray(np.asarray(dates, dtype=np.int32))
    W = np.asarray(W, dtype=np.float32)
    b = np.asarray(b, dtype=np.float32)

    # ragged lengths (same formula as the reference; host-side metadata)
    nz = dates != 0
    lengths = dates.shape[1] - 1 - np.argmax(nz[:, ::-1], axis=1)

    # fold the frozen sinusoid table through the decode weights (constants)
    tab = _sinusoid_table()
    ewc = (tab.astype(np.float64) @ W.astype(np.float64).T
           + b.astype(np.float64)).astype(np.float32)
    # date-pair table: row d1*TBASE+d2 = [EWc[d1] | EWc[d2]]
    ewc2 = np.empty((TBASE * TBASE, 2 * C), np.float32)
    ewc2[:, :C] = np.repeat(ewc, TBASE, axis=0)
    ewc2[:, C:] = np.tile(ewc, (TBASE, 1))

    aux = np.zeros((128, 10 + P + 128), np.float32)
    aux[:, 0:C] = W.T
    aux[:, C:C + P] = np.tile(np.eye(P, dtype=np.float32), (2, 1))
    aux[0, C + P:] = 1.0

    if "prog" not in _cache:
        _cache["prog"] = _build_program()
    nc = _cache["prog"]

    in_maps = []
    for core in range(NCORES):
        ds = dates[core * BPC:(core + 1) * BPC]
        idx = ds.reshape(BPC, 2, 128).transpose(2, 0, 1).reshape(128, NTILES)
        idx2 = np.ascontiguousarray(
            idx[:, 0::2].astype(np.int64) * TBASE + idx[:, 1::2]
        ).astype(np.int32)
        in_maps.append({
            "x_s": np.ascontiguousarray(x[core * BPC:(core + 1) * BPC]),
            "ewc2": ewc2,
            "idx2": idx2,
            "aux": aux,
        })

    res = bass_utils.run_bass_kernel_spmd(
        nc, in_maps, core_ids=list(range(NCORES)),
        trace=bool(os.environ.get("BASS_TRACE")),
    )
    LAST_RESULTS = res

    # unshard: slice each batch's ragged prefix from the padded output
    pieces = []
    for core in range(NCORES):
        o = res.results[core]["out"].reshape(BPC, TMAX, C, P)
        for i in range(BPC):
            pieces.append(o[i, :lengths[core * BPC + i]])
    recons = np.concatenate(pieces, axis=0)

    batch_idx = np.repeat(np.arange(B), lengths)
    time_idx = np.concatenate([np.arange(L) for L in lengths])
    masks = np.stack([batch_idx.astype(np.int32), time_idx.astype(np.int32)],
                     axis=1)
    return recons, masks
